# revision 24
# baseline (speedup 1.0000x reference)
"""DeepseekMoE (moe_routing) Trainium2 kernel.

Strategy (8 NeuronCores, single SPMD program):
  - Routing (grouped top-k; tiny T x H @ H x E) runs on host in numpy.
  - Routed experts are expert-parallel with load balancing: the per-expert
    token counts are highly skewed, so each core runs S slot classes with
    fixed capacities (chosen by a small search, e.g. (320, 176, 96)); an
    expert's token set may be SPLIT across several slot bins (on the same
    or different cores).  Every bin streams its expert's full w13/w2 panel
    set; tokens are gathered host-side into a transposed [H, C] activation
    block per bin; the device runs grouped GEMM1 -> SwiGLU -> GEMM2 per
    slot with the top-k combine weight folded into the GEMM2 PSUM evict.
  - Shared expert MLP is tensor-parallel over the 8 cores along the
    intermediate dim (2816 -> 8 x 352, zero-padded to 8 x 384).
  - All matmul operands are bf16 (cast host-side), accumulation f32.
  - Device returns per-slot y^T [H, C] (bf16) plus the shared partial
    [H, T] (bf16); host sums partials and scatter-adds slot outputs.
  - GEMM1 interleaves slot pairs (pair-major) and GEMM2 interleaves
    m-panels across slots so weight-panel DMA consumption stays close to
    the ~360 GB/s per-core HBM rate; the shared-expert phases (which use
    resident weights) act as DMA catch-up windows.
"""

import numpy as np
import ml_dtypes

import concourse.mybir as mybir
import concourse.tile as tile
from concourse import bacc
from concourse.bass_utils import run_bass_kernel_spmd

BF16 = ml_dtypes.bfloat16
F32 = np.float32

# Problem shapes (fixed by the spec).
T, H, E, I = 1024, 2048, 16, 1408
I2 = 2 * I                      # 2816 (w13 rows per expert)
IS = 2 * I                      # shared intermediate (n_shared=2 -> 2816)
SSH = 384                       # per-core shared shard (2816 padded to 3072 = 8*384)
TOP_K, N_GROUP, TOPK_GROUP = 4, 4, 2
ROUTED_SCALE = 2.5
N_CORES = 8
P = 128
KH = H // P                     # 16 K-subtiles over H
KI = I // P                     # 11 K-subtiles over I
MW = I2 // P                    # 22 M-panels over 2I
MH = H // P                     # 16 M-panels over H
NPAIR = I // P                  # 11 (g,u) SwiGLU pairs per slot
KS = SSH // P                   # 3 K-subtiles over shared shard


def _sigmoid(x):
    return 1.0 / (1.0 + np.exp(-x))


def _route(x, gate_weight, gate_bias):
    """Numpy port of reference._grouped_topk (float64 internally)."""
    logits = x.astype(np.float64) @ gate_weight.astype(np.float64).T
    scores = _sigmoid(logits)
    choice = scores + gate_bias.astype(np.float64)[None, :]
    g = choice.reshape(T, N_GROUP, E // N_GROUP)
    top2sum = np.sort(g, axis=-1)[..., -2:].sum(-1)          # [T, NG]
    gidx = np.argsort(-top2sum, axis=-1, kind="stable")[:, :TOPK_GROUP]
    gmask = np.zeros((T, N_GROUP), bool)
    gmask[np.arange(T)[:, None], gidx] = True
    emask = np.repeat(gmask, E // N_GROUP, axis=1)           # [T, E]
    masked = np.where(emask, choice, -np.inf)
    topk_ids = np.argsort(-masked, axis=-1, kind="stable")[:, :TOP_K]
    topk_w = np.take_along_axis(scores, topk_ids, axis=1)
    topk_w = topk_w / topk_w.sum(-1, keepdims=True) * ROUTED_SCALE
    return topk_ids.astype(np.int32), topk_w


def _pack_lhs_panels(w, n_m, n_k):
    """[n_m*128, n_k*128] (indexed [M, K]) -> [n_m, 128, n_k, 128] panels
    where panel[m][p, k, j] = w[128*m + j, 128*k + p], i.e. each panel
    slice [:, k, :] is the lhsT chunk [K-sub=128, M-sub=128]."""
    a = w.reshape(n_m, P, n_k, P)          # [m, j, k, p]
    return np.ascontiguousarray(a.transpose(0, 3, 2, 1))


def _pack_rhs(xcols):
    """[C, H] token-major rows -> [128, KH, C] rhs layout:
    out[p, k, c] = xcols[c, 128*k + p]."""
    a = xcols.reshape(-1, KH, P)           # [c, k, p]
    return np.ascontiguousarray(a.transpose(2, 1, 0))


def _nchunks(c):
    out = []
    o = 0
    while o < c:
        n = min(512, c - o)
        out.append((o, n))
        o += n
    return out


# --------------------------------------------------------------------------
# Slot planning: pick S<=3 slot capacities (x8 bins each) and split the
# experts' token sets across bins so every core does the same amount of
# work.  Score = max(PE estimate, DMA estimate) for the ridge regime.
# --------------------------------------------------------------------------

def _try_caps(sizes, caps):
    """Best-fit-decreasing with splitting.  sizes: [(expert, count)] desc;
    caps: desc list of capacities (8 bins each).  Returns list of
    (class_idx, expert, start, n) or None."""
    avail = [8] * len(caps)
    bins = []
    for e, c in sizes:
        start = 0
        rem = c
        while rem > 0:
            cands = [i for i in range(len(caps)) if avail[i] > 0]
            if not cands:
                return None
            fit = [i for i in cands if caps[i] >= rem]
            i = max(fit) if fit else min(cands)   # caps desc: max idx = smallest fitting
            take = min(rem, caps[i])
            avail[i] -= 1
            bins.append((i, e, start, take))
            start += take
            rem -= take
    return bins


def _plan_slots(counts):
    """counts: int array [E].  Returns (caps tuple, bins list)."""
    total = int(counts.sum())
    cmax = int(counts.max())
    order = np.argsort(-counts, kind="stable")
    sizes = [(int(e), int(counts[e])) for e in order if counts[e] > 0]
    r8 = lambda v: max(8, -(-int(v) // 8) * 8)

    def score(caps):
        pe = (528 * sum(caps) + 147456) / 2.4
        dma = (len(caps) * 17.3e6 + 3.1e6 + sum(caps) * H * 2 + 4.2e6) / 0.36e3
        return max(pe, dma)

    best = None
    for C1 in range(r8(cmax / 3), r8(cmax) + 8, 8):
        for C2 in range(8, C1 + 8, 8):
            for C3 in range(0, C2 + 8, 8):
                caps = tuple(c for c in (C1, C2, C3) if c > 0)
                if 8 * sum(caps) < total:
                    continue
                sc = score(caps)
                if best is not None and sc >= best[0]:
                    continue
                bins = _try_caps(sizes, caps)
                if bins is not None:
                    best = (sc, caps, bins)
    if best is None:
        caps = (r8(cmax),) * 3
        return caps, _try_caps(sizes, caps)
    return best[1], best[2]


_PROGRAM_CACHE = {}


def _get_program(caps):
    if caps not in _PROGRAM_CACHE:
        _PROGRAM_CACHE[caps] = _build_program(caps)
    return _PROGRAM_CACHE[caps]


def _build_program(caps, reps=1, hw_loop=True):
    """One SPMD Tile program shared by all 8 cores. caps: slot capacities.
    reps>1 wraps the compute in a hardware loop (timing amplification);
    hw_loop=False unrolls instead (for the timeline simulator)."""
    nc = bacc.Bacc(None, target_bir_lowering=False)
    bf = mybir.dt.bfloat16
    f32 = mybir.dt.float32
    ns = len(caps)

    # --- I/O ----------------------------------------------------------
    w13q = [nc.dram_tensor(f"w13q{s}", [MW, P, KH, P], bf, kind="ExternalInput")
            for s in range(ns)]
    w2q = [nc.dram_tensor(f"w2q{s}", [MH, P, KI, P], bf, kind="ExternalInput")
           for s in range(ns)]
    xgq = [nc.dram_tensor(f"xgq{s}", [P, KH, caps[s]], bf, kind="ExternalInput")
           for s in range(ns)]
    wtb = [nc.dram_tensor(f"wtb{s}", [P, caps[s]], f32, kind="ExternalInput")
           for s in range(ns)]
    sguq = nc.dram_tensor("sguq", [2 * KS, P, KH, P], bf, kind="ExternalInput")
    sdq = nc.dram_tensor("sdq", [MH, P, KS, P], bf, kind="ExternalInput")
    xtq = nc.dram_tensor("xtq", [P, KH, T], bf, kind="ExternalInput")

    yout = [nc.dram_tensor(f"y{s}", [MH, P, caps[s]], bf, kind="ExternalOutput")
            for s in range(ns)]
    shp = nc.dram_tensor("shp", [MH, P, T], bf, kind="ExternalOutput")

    with tile.TileContext(nc) as tc:
        with (
            tc.tile_pool(name="resident", bufs=1) as res,
            tc.tile_pool(name="wpanel1", bufs=12) as wpool1,
            tc.tile_pool(name="wpanel2", bufs=12) as wpool2,
            tc.tile_pool(name="hbuf", bufs=2) as hpool,
            tc.tile_pool(name="silu", bufs=4) as spool,
            tc.tile_pool(name="outbuf", bufs=4) as opool,
            tc.tile_pool(name="psum", bufs=8, space="PSUM") as psum1,
        ):
            # Resident activations (loaded once, outside the reps loop)
            xg_t, wt_t = [], []
            for s in range(ns):
                c = caps[s]
                t = res.tile([P, KH, c], bf, name=f"xg{s}_t")
                nc.sync.dma_start(t[:], xgq[s].ap()[:])
                xg_t.append(t)
                w = res.tile([P, c], f32, name=f"wt{s}_t")
                nc.sync.dma_start(w[:], wtb[s].ap()[:])
                wt_t.append(w)
            xt_t = res.tile([P, KH, T], bf)
            nc.sync.dma_start(xt_t[:], xtq.ap()[:])
            sd_t = res.tile([P, KS, H], bf)   # resident shared-down panels
            for m in range(MH):
                nc.sync.dma_start(sd_t[:, :, m * P:(m + 1) * P], sdq.ap()[m])

            def pair_g1(wq_ap, mg, mu, rhs_t, n_k, cap, h_out, pr, wpool, wtag):
                """One SwiGLU pair: h_out[:, pr, :] = silu(g) * u with
                g = W[mg] @ x, u = W[mu] @ x."""
                panels, psums = [], []
                for m in (mg, mu):
                    pan = wpool.tile([P, KH, P], bf, tag=wtag)
                    nc.sync.dma_start(pan[:, :n_k, :], wq_ap[m])
                    panels.append(pan)
                    ps = [psum1.tile([P, 512], mybir.dt.float32, tag="ps",
                                     name=f"ps_g1_{wtag}_{pr}_{m}_{ci}")
                          for ci in range(len(_nchunks(cap)))]
                    for k in range(n_k):
                        for ci, (o, n) in enumerate(_nchunks(cap)):
                            nc.tensor.matmul(
                                ps[ci][:, :n],
                                lhsT=pan[:, k, :],
                                rhs=rhs_t[:, k, o:o + n],
                                start=(k == 0),
                                stop=(k == n_k - 1),
                            )
                    psums.append(ps)
                for ci, (o, n) in enumerate(_nchunks(cap)):
                    # silu(g) * u as sigmoid(g) * g * u (Silu itself is
                    # not implemented in CoreSim).
                    sg = spool.tile([P, 512], mybir.dt.float32, tag="sg")
                    nc.scalar.activation(
                        sg[:, :n], psums[0][ci][:, :n],
                        mybir.ActivationFunctionType.Sigmoid,
                    )
                    nc.vector.tensor_mul(
                        sg[:, :n], sg[:, :n], psums[0][ci][:, :n],
                    )
                    nc.vector.tensor_mul(
                        h_out[:, pr, o:o + n], sg[:, :n], psums[1][ci][:, :n],
                    )

            # Insert shared-G1 pair j after routed pr-group i: the shared
            # pairs consume almost no panel DMA, letting the w13 stream
            # catch back up to the (faster-than-DMA) routed consumption.
            SH_AFTER = {3: 0, 7: 1, 10: 2}

            def g2_routed_m(s, m, h_t):
                cap = caps[s]
                pan = wpool2.tile([P, KI, P], bf, tag="wpanel2")
                # issue on the idle Pool queue so the w2 ring pre-fills
                # during GEMM1 instead of queueing behind w13 on SP
                nc.gpsimd.dma_start(pan[:, :KI, :], w2q[s].ap()[m])
                ps = [psum1.tile([P, 512], mybir.dt.float32, tag="ps",
                                 name=f"ps_g2_{s}_{m}_{ci}")
                      for ci in range(len(_nchunks(cap)))]
                for k in range(KI):
                    for ci, (o, n) in enumerate(_nchunks(cap)):
                        nc.tensor.matmul(
                            ps[ci][:, :n],
                            lhsT=pan[:, k, :],
                            rhs=h_t[s][:, k, o:o + n],
                            start=(k == 0),
                            stop=(k == KI - 1),
                        )
                ot = opool.tile([P, cap], bf, tag=f"yout{s}")
                for ci, (o, n) in enumerate(_nchunks(cap)):
                    nc.vector.tensor_mul(
                        ot[:, o:o + n], ps[ci][:, :n], wt_t[s][:, o:o + n],
                    )
                nc.scalar.dma_start(yout[s].ap()[m], ot[:])

            def body():
                h_t = [hpool.tile([P, KI, caps[s]], bf, name=f"h{s}_t",
                                  tag=f"h{s}_t") for s in range(ns)]
                hs_t = hpool.tile([P, KS, T], bf, tag="hs_t")
                # --- GEMM1: routed pair-major across slots, shared pairs
                # interleaved as DMA catch-up windows ---------------------
                for pr in range(NPAIR):
                    for s in range(ns):
                        pair_g1(w13q[s].ap(), pr, pr + NPAIR, xg_t[s], KH,
                                caps[s], h_t[s], pr, wpool1, "wpanel1")
                    if pr in SH_AFTER:
                        j = SH_AFTER[pr]
                        pair_g1(sguq.ap(), j, j + KS, xt_t, KH, T, hs_t, j,
                                wpool1, "wpanel1")
                # --- GEMM2: shared m-panel (sd resident, zero weight DMA)
                # interleaved with routed m-panels ------------------------
                for m in range(MH):
                    ps = [psum1.tile([P, 512], mybir.dt.float32, tag="ps",
                                     name=f"ps_sh_{m}_{ci}")
                          for ci in range(len(_nchunks(T)))]
                    for k in range(KS):
                        for ci, (o, n) in enumerate(_nchunks(T)):
                            nc.tensor.matmul(
                                ps[ci][:, :n],
                                lhsT=sd_t[:, k, m * P:(m + 1) * P],
                                rhs=hs_t[:, k, o:o + n],
                                start=(k == 0),
                                stop=(k == KS - 1),
                            )
                    ot = opool.tile([P, T], bf, tag="shout")
                    for ci, (o, n) in enumerate(_nchunks(T)):
                        nc.any.tensor_copy(ot[:, o:o + n], ps[ci][:, :n])
                    # output writes issue on the Activation queue (idle in
                    # GEMM2) so SP is free to prefetch next-rep w13 panels
                    nc.scalar.dma_start(shp.ap()[m], ot[:])
                    for s in range(ns):
                        g2_routed_m(s, m, h_t)

            if reps == 1:
                body()
            elif not hw_loop:
                for _ in range(reps):
                    body()
            else:
                with tc.For_i(0, reps, 1):
                    body()

    nc.compile()
    return nc


def _prepare(x, gate_weight, gate_bias, w13, w2, shared_gate_up, shared_down):
    """Host-side routing + slot planning + packing.
    Returns (caps, in_maps, meta)."""
    topk_ids, topk_w = _route(x, gate_weight, gate_bias)
    flat_e = topk_ids.ravel()
    flat_w = topk_w.ravel()
    flat_t = np.repeat(np.arange(T, dtype=np.int64), TOP_K)
    idx_e = [flat_t[flat_e == e] for e in range(E)]
    w_e = [flat_w[flat_e == e] for e in range(E)]
    counts = np.array([len(i) for i in idx_e])

    caps, bins = _plan_slots(counts)
    ns = len(caps)
    # distribute bins to (class, core): class i bins fill cores 0..7 in order
    percore = [[None] * ns for _ in range(N_CORES)]
    nextcore = [0] * ns
    for (i, e, start, take) in bins:
        c = nextcore[i]
        nextcore[i] += 1
        percore[c][i] = (e, start, take)

    xt_pack = _pack_rhs(x.astype(BF16))                 # [128, KH, T]
    w13_bf = [None] * E
    w2_bf = [None] * E

    zero_w13 = None
    zero_w2 = None

    in_maps, meta = [], []
    for c in range(N_CORES):
        im = {}
        cmeta = []
        for s in range(ns):
            cap = caps[s]
            entry = percore[c][s]
            if entry is None:
                if zero_w13 is None:
                    zero_w13 = np.zeros((MW, P, KH, P), dtype=BF16)
                    zero_w2 = np.zeros((MH, P, KI, P), dtype=BF16)
                im[f"xgq{s}"] = np.zeros((P, KH, cap), dtype=BF16)
                im[f"wtb{s}"] = np.zeros((P, cap), dtype=F32)
                im[f"w13q{s}"] = zero_w13
                im[f"w2q{s}"] = zero_w2
                continue
            e, start, n = entry
            idx = idx_e[e][start:start + n]
            xg = np.zeros((cap, H), dtype=BF16)
            xg[:n] = x[idx].astype(BF16)
            im[f"xgq{s}"] = _pack_rhs(xg)
            wt = np.zeros((cap,), dtype=F32)
            wt[:n] = w_e[e][start:start + n].astype(F32)
            im[f"wtb{s}"] = np.ascontiguousarray(
                np.broadcast_to(wt[None, :], (P, cap)).astype(F32))
            if w13_bf[e] is None:
                w13_bf[e] = _pack_lhs_panels(w13[e].astype(BF16), MW, KH)
                w2_bf[e] = _pack_lhs_panels(w2[e].astype(BF16), MH, KI)
            im[f"w13q{s}"] = w13_bf[e]
            im[f"w2q{s}"] = w2_bf[e]
            cmeta.append((s, e, idx))
        # shared shard: rows [c*352, (c+1)*352) of gate and up, padded to 384
        sh = IS // N_CORES
        lo, hi = c * sh, (c + 1) * sh
        gsl = np.zeros((SSH, H), dtype=F32)
        usl = np.zeros((SSH, H), dtype=F32)
        gsl[:hi - lo] = shared_gate_up[lo:hi]
        usl[:hi - lo] = shared_gate_up[IS + lo:IS + hi]
        sgu_pad = np.concatenate([gsl, usl], 0).astype(BF16)   # [768, H]
        im["sguq"] = _pack_lhs_panels(sgu_pad, 2 * KS, KH)
        sd_sl = np.zeros((H, SSH), dtype=F32)
        sd_sl[:, :hi - lo] = shared_down[:, lo:hi]
        im["sdq"] = _pack_lhs_panels(sd_sl.astype(BF16), MH, KS)
        im["xtq"] = xt_pack
        in_maps.append(im)
        meta.append(cmeta)
    return caps, in_maps, meta


def _combine(results, meta):
    out = np.zeros((H, T), dtype=F32)
    for c in range(N_CORES):
        out += results[c]["shp"].reshape(H, T).astype(F32)
    out = np.ascontiguousarray(out.T)                   # [T, H]
    for c in range(N_CORES):
        r = results[c]
        for (s, e, idx) in meta[c]:
            n = len(idx)
            if n:
                y = r[f"y{s}"].reshape(H, -1).astype(F32)   # [H, cap]
                np.add.at(out, idx, y[:, :n].T)
    return out


def kernel(hidden_states, gate_weight, gate_bias, w13, w2,
           shared_gate_up, shared_down):
    x = np.asarray(hidden_states, dtype=F32)
    gate_weight = np.asarray(gate_weight, dtype=F32)
    gate_bias = np.asarray(gate_bias, dtype=F32)
    w13 = np.asarray(w13, dtype=F32)
    w2 = np.asarray(w2, dtype=F32)
    shared_gate_up = np.asarray(shared_gate_up, dtype=F32)
    shared_down = np.asarray(shared_down, dtype=F32)

    caps, in_maps, meta = _prepare(
        x, gate_weight, gate_bias, w13, w2, shared_gate_up, shared_down)
    nc = _get_program(caps)
    res = run_bass_kernel_spmd(nc, in_maps, core_ids=list(range(N_CORES)))
    return _combine(res.results, meta)
